# revision 15
# baseline (speedup 1.0000x reference)
"""YOLOv1 loss kernel for 8 Trainium2 NeuronCores.

Strategy (data-parallel, per spec sharding hint):
  - Shard the batch dim (32768) across 8 cores -> 4096 samples/core.
  - Each core computes a per-partition partial sum of the loss terms; the
    host does the final (tiny) reduction across 8*128*2 floats in float64.

Math notes (validated against the jax reference):
  - The grid offsets (m, n) cancel inside the IoU (all box corners share the
    same +m/G, +n/G shift), so no iota/grid constants are needed.
  - IoU is computed in 7x-scaled coordinates: corners c +/- 3.5w, areas
    scaled by 49: iou = i7/u7 with i7 = relu(iw)*relu(ih), u7 = 49*(ap+ag)-i7.
  - where(inter>0, inter/union, 0) is automatic: i7 == 0 -> iou == 0.
  - total = sum(obj*(sel + cls - 0.5*nq)) + 0.5*sum(nq),  nq = p4^2 + p9^2.

Engine split (v2, tuned from the NTFF profile of v1):
  - DVE runs tensor_tensor / tensor_scalar ops in bf16 (2x packed mode,
    ~1.9 elem/cycle measured); the 1x-only scalar_tensor_tensor ops were
    eliminated except the fp32 union term.
  - ACT runs the LUT ops (sqrt, square, relu) plus the scale-by-constant
    copies (3.5*wh), using its idle capacity.
  - The reciprocal stays the fp32 DVE approx (ACT Reciprocal is blocked).

Layout: partition = sample block (128), free = [samples(8), channels, cells].
"""

import numpy as np

import concourse.bacc as bacc
import concourse.tile as tile
from concourse import mybir
from concourse.bass_utils import run_bass_kernel_spmd

# Problem constants (hardcoded per contract; kernel.py must be self-contained).
B = 32768
N_CORES = 8
BC = B // N_CORES            # 4096 samples per core
P = 128                      # SBUF partitions
S = 8                        # samples per partition per block
NBLK = BC // (P * S)         # 4 blocks per core
K = 49                       # grid cells (7*7)

F32 = mybir.dt.float32
BF16 = mybir.dt.bfloat16

AL = mybir.AluOpType
AF = mybir.ActivationFunctionType


def _build(nblk=NBLK):
    nc = bacc.Bacc("TRN2", target_bir_lowering=False, debug=False,
                   num_devices=N_CORES)
    bc = nblk * P * S
    pred = nc.dram_tensor("pred", [bc, 30, K], F32, kind="ExternalInput")
    labels = nc.dram_tensor("labels", [bc, 30, K], F32, kind="ExternalInput")
    out = nc.dram_tensor("acc", [P, 2], F32, kind="ExternalOutput")

    pred_r = pred.ap().rearrange("(t p s) c k -> t p s c k", p=P, s=S)
    lab_r = labels.ap().rearrange("(t p s) c k -> t p s c k", p=P, s=S)

    with tile.TileContext(nc) as tc:
        with (
            tc.tile_pool(name="io", bufs=2) as io,
            tc.tile_pool(name="quadp", bufs=10) as wk,
            tc.tile_pool(name="bip", bufs=10) as bip,
            tc.tile_pool(name="unitp", bufs=12) as unitp,
            tc.tile_pool(name="ufp", bufs=3) as ufp,
            tc.tile_pool(name="treep", bufs=2) as treep,
            tc.tile_pool(name="accp", bufs=1) as accp,
        ):
            ACC = accp.tile([P, S, K], F32, tag="ACC")
            ACCN = accp.tile([P, S, K], F32, tag="ACCN")
            nc.vector.memset(ACC, 0.0)
            nc.vector.memset(ACCN, 0.0)

            ctxs = []
            for t in range(nblk):
                ctxs.append(_block_box(nc, io, wk, bip, unitp, ufp,
                                       pred_r[t], lab_r[t]))
                if t >= 1:
                    _block_cls_tail(nc, bip, unitp, treep, ACC, ACCN,
                                    ctxs[t - 1])
            _block_cls_tail(nc, bip, unitp, treep, ACC, ACCN, ctxs[-1])

            # ---- final per-core reduce: [P,S,K] -> [P,1] each ----
            red = accp.tile([P, 2], F32, tag="red")
            nc.vector.tensor_reduce(out=red[:, 0:1], in_=ACC[:],
                                    axis=mybir.AxisListType.XY, op=AL.add)
            nc.vector.tensor_reduce(out=red[:, 1:2], in_=ACCN[:],
                                    axis=mybir.AxisListType.XY, op=AL.add)
            nc.sync.dma_start(out=out.ap(), in_=red)

    nc.finalize()
    return nc


def _block_box(nc, io, wk, bip, unitp, ufp, pred_t, lab_t):
    """Box/IoU/conf work for one block of P*S samples, ending with the cls
    subtract (whose lcls input is the last DMA of the block) and the cls
    square on ACT. The cls reduction tree is deferred to _block_cls_tail so
    the next block's independent box work can fill the DVE queue while this
    block's cls data finishes landing."""
    import concourse.bass as bass

    def box_view(base):
        # [P, S, 2, 6, K]: box b reads channels {5b..5b+5} (one-channel overlap
        # pads each box block to 6 channels so quad views keep even strides).
        ap = [list(x) for x in base.ap]
        return bass.AP(tensor=base.tensor, offset=base.offset,
                       ap=[ap[0], ap[1], [5 * K, 2], [K, 6], [1, K]])

    # ---- input DMAs (SWDGE cast fp32 -> bf16) ----
    pbox = io.tile([P, S, 2, 6, K], BF16, tag="pbox", name="pbox")
    lbox = io.tile([P, S, 2, 6, K], BF16, tag="lbox", name="lbox")
    pcls = io.tile([P, S, 20, K], BF16, tag="pcls", bufs=3, name="pcls")
    lcls = io.tile([P, S, 20, K], BF16, tag="lcls", name="lcls")
    nc.gpsimd.dma_start(out=pbox, in_=box_view(pred_t))
    nc.gpsimd.dma_start(out=lbox, in_=box_view(lab_t))
    nc.gpsimd.dma_start(out=pcls, in_=pred_t[:, :, 10:30, :])
    nc.gpsimd.dma_start(out=lcls, in_=lab_t[:, :, 10:30, :])

    pb, lb = pbox[:], lbox[:]
    p_c = pb[:, :, :, 0:2, :]       # pred centers x,y  (quad [P,S,2,2,K])
    p_wh = pb[:, :, :, 2:4, :]      # pred w,h          (quad)
    p49 = pb[:, :, :, 4, :]         # conf p4,p9        (bi [P,S,2,K])
    l_c = lb[:, :, 0, 0:2, :]       # gt centers        (pair [P,S,2,K])
    l_wh = lb[:, :, 0, 2:4, :]      # gt w,h            (pair)
    l_c56 = lb[:, :, :, 0:2, :]     # labels ch{0,1,5,6} (quad)
    l_wh78 = lb[:, :, :, 2:4, :]    # labels ch{2,3,7,8} (quad)
    l4 = lb[:, :, 0, 4, :]          # obj mask          (unit [P,S,K])

    QUAD, BI = (P, S, 2, 2, K), (P, S, 2, K)

    def qt(tag="quad"):
        return wk.tile([P, S, 2, 2, K], BF16, tag="quad", name=f"q_{tag}")

    def bt(tag="bi"):
        return bip.tile([P, S, 2, K], BF16, tag="bi", name=f"b_{tag}")

    def ut(tag="unit", bufs=None):
        return unitp.tile([P, S, K], BF16, tag=tag if bufs else "unit",
                          bufs=bufs, name=f"u_{tag}")

    # ---- IoU: corners via ACT-scaled half-extents, min/max on DVE ----
    w35p = qt("w35p")
    nc.scalar.activation(out=w35p, in_=p_wh, func=AF.Copy, scale=3.5)
    w35g = bt("w35g")
    nc.scalar.activation(out=w35g, in_=l_wh, func=AF.Copy, scale=3.5)
    lo, hi = qt(), qt()
    nc.vector.tensor_sub(out=lo, in0=p_c, in1=w35p)
    nc.vector.tensor_add(out=hi, in0=p_c, in1=w35p)
    glo, ghi = bt(), bt()
    nc.vector.tensor_sub(out=glo, in0=l_c, in1=w35g)
    nc.vector.tensor_add(out=ghi, in0=l_c, in1=w35g)
    glo_b = glo[:].unsqueeze(2).to_broadcast(QUAD)
    ghi_b = ghi[:].unsqueeze(2).to_broadcast(QUAD)
    mins, maxs = qt(), qt()
    nc.vector.tensor_tensor(out=mins, in0=hi, in1=ghi_b, op=AL.min)
    nc.vector.tensor_tensor(out=maxs, in0=lo, in1=glo_b, op=AL.max)
    dd = qt()
    nc.vector.tensor_sub(out=dd, in0=mins, in1=maxs)
    dch = qt("dch")
    nc.scalar.activation(out=dch, in_=dd, func=AF.Relu)

    i4 = bt("i4")
    nc.vector.tensor_mul(out=i4, in0=dch[:, :, :, 0, :], in1=dch[:, :, :, 1, :])
    m = bt()
    nc.vector.tensor_mul(out=m, in0=p_wh[:, :, :, 0, :], in1=p_wh[:, :, :, 1, :])
    mg = unitp.tile([P, S, K], BF16, tag="unit", name="u_mg")
    nc.vector.tensor_mul(out=mg, in0=l_wh[:, :, 0, :], in1=l_wh[:, :, 1, :])
    msum = bt()
    nc.vector.tensor_add(out=msum, in0=m,
                         in1=mg[:].unsqueeze(2).to_broadcast(BI))
    # union in fp32 (ACT Reciprocal is blocked in bass; DVE approx needs fp32)
    u = ufp.tile([P, S, 2, K], F32, tag="uf", name="b_u")
    nc.vector.scalar_tensor_tensor(out=u, in0=msum, scalar=49.0, in1=i4,
                                   op0=AL.mult, op1=AL.subtract)
    r = ufp.tile([P, S, 2, K], F32, tag="uf", name="b_r")
    nc.vector.reciprocal_approx_fast(
        out=r[:].rearrange("p s b k -> p (s b k)"),
        in_=u[:].rearrange("p s b k -> p (s b k)"))
    iou = bt("iou")
    nc.vector.tensor_mul(out=iou, in0=i4, in1=r)

    # ---- coordinate loss ----
    d = qt("d")
    nc.vector.tensor_sub(out=d, in0=p_c, in1=l_c56)
    sp, sl = qt("sp"), qt("sl")
    nc.scalar.sqrt(out=sp, in_=p_wh)
    nc.scalar.sqrt(out=sl, in_=l_wh78)
    dsq = qt()
    nc.vector.tensor_sub(out=dsq, in0=sp, in1=sl)
    sqd, sqds = qt(), qt()
    nc.scalar.square(out=sqd, in_=d)
    nc.scalar.square(out=sqds, in_=dsq)
    s12 = qt()
    nc.vector.tensor_add(out=s12, in0=sqd, in1=sqds)
    tab = bt("tab")
    nc.vector.tensor_add(out=tab, in0=s12[:, :, :, 0, :], in1=s12[:, :, :, 1, :])

    # ---- confidence + selection ----
    e49 = bt()
    nc.vector.tensor_sub(out=e49, in0=p49, in1=iou)
    esq = bt("esq")
    nc.scalar.square(out=esq, in_=e49)
    x5 = bt()
    nc.vector.tensor_scalar_mul(out=x5, in0=tab, scalar1=5.0)
    x = bt("x")
    nc.vector.tensor_add(out=x, in0=x5, in1=esq)
    he = bt("he")
    nc.vector.tensor_scalar_mul(out=he, in0=esq, scalar1=0.5)
    lb1, lb2 = ut("lb1"), ut("lb2")
    nc.vector.tensor_add(out=lb1, in0=x[:, :, 0, :], in1=he[:, :, 1, :])
    nc.vector.tensor_add(out=lb2, in0=x[:, :, 1, :], in1=he[:, :, 0, :])
    resp = ut("resp")
    nc.vector.tensor_tensor(out=resp, in0=iou[:, :, 0, :], in1=iou[:, :, 1, :],
                            op=AL.is_ge)
    dlb = ut("dlb")
    nc.vector.tensor_sub(out=dlb, in0=lb1, in1=lb2)
    sd = ut("sd")
    nc.vector.tensor_mul(out=sd, in0=dlb, in1=resp)
    sel = ut("sel")
    nc.vector.tensor_add(out=sel, in0=lb2, in1=sd)

    sq49 = bt()
    nc.scalar.square(out=sq49, in_=p49)
    nq = ut("nq", bufs=2)
    nc.vector.tensor_add(out=nq, in0=sq49[:, :, 0, :], in1=sq49[:, :, 1, :])
    nqh = ut("nqh")
    nc.vector.tensor_scalar_mul(out=nqh, in0=nq, scalar1=0.5)
    w1 = ut("w1", bufs=2)
    nc.vector.tensor_sub(out=w1, in0=sel, in1=nqh)
    mask = ut("mask", bufs=2)
    nc.vector.tensor_single_scalar(out=mask, in_=l4, scalar=1.0, op=AL.is_equal)

    # ---- cls subtract: lcls is this block's last DMA, landing right as the
    # DVE reaches this op; square runs on ACT at the end of its stream ----
    nc.vector.tensor_sub(out=pcls, in0=pcls, in1=lcls)
    nc.scalar.square(out=pcls, in_=pcls)

    return {"pcls": pcls, "w1": w1, "mask": mask, "nq": nq}


def _block_cls_tail(nc, bip, unitp, treep, ACC, ACCN, ctx):
    """Cls reduction tree + combine for a block whose box part already ran."""
    pcls, w1, mask, nq = ctx["pcls"], ctx["w1"], ctx["mask"], ctx["nq"]

    def ut(tag="unit"):
        return unitp.tile([P, S, K], BF16, tag="unit", name=f"u_{tag}")

    ta = treep.tile([P, S, 10, K], BF16, tag="ta", name="ta")
    nc.vector.tensor_add(out=ta, in0=pcls[:, :, 0:10, :], in1=pcls[:, :, 10:20, :])
    tb = treep.tile([P, S, 4, K], BF16, tag="tb", name="tb")
    nc.vector.tensor_add(out=tb, in0=ta[:, :, 0:4, :], in1=ta[:, :, 4:8, :])
    tc2 = bip.tile([P, S, 2, K], BF16, tag="bi", name="b_tc2")
    nc.vector.tensor_add(out=tc2, in0=tb[:, :, 0:2, :], in1=tb[:, :, 2:4, :])
    td = ut()
    nc.vector.tensor_add(out=td, in0=tc2[:, :, 0, :], in1=tc2[:, :, 1, :])
    te = ut()
    nc.vector.tensor_add(out=te, in0=ta[:, :, 8, :], in1=ta[:, :, 9, :])
    clsc = ut("clsc")
    nc.vector.tensor_add(out=clsc, in0=td, in1=te)

    w2 = ut()
    nc.vector.tensor_add(out=w2, in0=clsc, in1=w1)
    wm = ut()
    nc.vector.tensor_mul(out=wm, in0=w2, in1=mask)
    nc.vector.tensor_add(out=ACC, in0=ACC, in1=wm)
    nc.vector.tensor_add(out=ACCN, in0=ACCN, in1=nq)


_NC_CACHE = None


def _get_nc():
    global _NC_CACHE
    if _NC_CACHE is None:
        _NC_CACHE = _build()
    return _NC_CACHE


def _make_in_maps(pred: np.ndarray, labels: np.ndarray):
    pred = np.ascontiguousarray(pred, dtype=np.float32).reshape(B, 30, K)
    labels = np.ascontiguousarray(labels, dtype=np.float32).reshape(B, 30, K)
    in_maps = []
    for i in range(N_CORES):
        sl = slice(i * BC, (i + 1) * BC)
        in_maps.append({"pred": pred[sl], "labels": labels[sl]})
    return in_maps


def _reduce_results(results) -> np.ndarray:
    total = np.float64(0.0)
    for i in range(N_CORES):
        acc = results[i]["acc"].astype(np.float64)
        total += acc[:, 0].sum() + 0.5 * acc[:, 1].sum()
    return np.asarray(np.float32(total / B))


def kernel(pred: np.ndarray, labels: np.ndarray) -> np.ndarray:
    nc = _get_nc()
    in_maps = _make_in_maps(pred, labels)
    res = run_bass_kernel_spmd(nc, in_maps, core_ids=list(range(N_CORES)),
                               trace=False)
    return _reduce_results(res.results)
